# revision 19
# baseline (speedup 1.0000x reference)
"""Trainium2 Bass kernel for nn_MemModule (sparse attention memory module).

Computes, for z [16384, 1024] f32 and memory [4096, 1024] f32:
    w   = softmax(z @ memory.T, axis=1)
    wh  = relu(w - lam) * w / (|w - lam| + eps)   # lam = 2/4096, eps = 1e-12
    att = softmax(wh, axis=1)
    out = (att, att @ memory)

Strategy (data-parallel over batch on 8 NeuronCores, memory replicated):

* No max-subtraction: logits are O(+-5), exp is safe; softmax is shift
  invariant so results match the reference to fp32 rounding.
* Shrink threshold applied in u-space (u = exp(logit)): w > lam is
  exactly u > lam * s with s = rowsum(u).  The eps-smoothing band of the
  reference (|w - lam| < ~1e-5) affects att by < 5e-4 relative on a
  handful of elements, so the hard step u > theta is used.
* Key small-number algebra: wh <= max(w) ~ 2.2e-3, so
  exp(wh) = 1 + wh to 2.4e-6 relative.  Therefore with v = wh:
      att  = (1 + v) / s2,        s2 = M + sum_j(v)
      zhat = (v @ memory + colsum(memory)) / s2
  The v-decomposition avoids the catastrophic bf16 cancellation that
  would occur when multiplying the nearly-uniform att by memory.
* GEMMs run in bf16 (error ~4e-3 on logits -> ~1e-5 relative on att,
  ~3e-3 on zhat modulation).  v is transposed for GEMM2 with the DMA
  xbar transpose (bf16, SBUF->SBUF).
"""

import os
import sys

import numpy as np

_PATHS = [
    "/root/.axon_site",
    "/root/.axon_site/_ro/trn_rl_repo",
    "/root/.axon_site/_ro/pypackages",
    "/opt/trn_rl_repo",
]
for _p in _PATHS:
    if os.path.isdir(_p) and _p not in sys.path:
        sys.path.append(_p)

import ml_dtypes  # noqa: E402

import concourse.bass as bass  # noqa: E402
import concourse.mybir as mybir  # noqa: E402
import concourse.tile as tile  # noqa: E402
from concourse import bacc  # noqa: E402
from concourse.bass_utils import run_bass_kernel_spmd  # noqa: E402

BF16 = ml_dtypes.bfloat16
F32 = np.float32

B = 16384       # batch
F = 1024        # feature dim
M = 4096        # mem dim
NCORES = 8
BL = B // NCORES          # rows per core = 2048
T = BL // 128             # row tiles per core = 16
KC = F // 128             # k chunks = 8
JC = M // 128             # j chunks = 32
LAM = 2.0 / M

_STATE: dict = {}


def _build_module():
    nc = bacc.Bacc(
        "TRN2",
        target_bir_lowering=False,
        debug=False,
        enable_asserts=True,
        num_devices=NCORES,
    )
    dt = mybir.dt
    afunc = mybir.ActivationFunctionType
    op = mybir.AluOpType

    zt_d = nc.dram_tensor("zt", [T, 128, KC, 128], dt.bfloat16, kind="ExternalInput").ap()
    memt_d = nc.dram_tensor("memt", [128, KC, M], dt.bfloat16, kind="ExternalInput").ap()
    memb_d = nc.dram_tensor("memb", [128, JC, F], dt.bfloat16, kind="ExternalInput").ap()
    colsum_d = nc.dram_tensor("colsum", [128, F], dt.float32, kind="ExternalInput").ap()
    att_d = nc.dram_tensor("att", [BL, M], dt.float32, kind="ExternalOutput").ap()
    zhat_d = nc.dram_tensor("zhat", [BL, F], dt.float32, kind="ExternalOutput").ap()

    with tile.TileContext(nc) as tc:
        with (
            tc.tile_pool(name="const", bufs=1) as const_pool,
            tc.tile_pool(name="zt", bufs=2) as zt_pool,
            tc.tile_pool(name="u", bufs=2) as u_pool,
            tc.tile_pool(name="v", bufs=2) as v_pool,
            tc.tile_pool(name="vt", bufs=2) as vt_pool,
            tc.tile_pool(name="att", bufs=2) as att_pool,
            tc.tile_pool(name="zh", bufs=2) as zh_pool,
            tc.tile_pool(name="stat", bufs=4) as stat_pool,
            tc.tile_pool(name="psw", bufs=2, space="PSUM") as psw_pool,
            tc.tile_pool(name="psz", bufs=2, space="PSUM") as psz_pool,
        ):
            # ---- resident constants (per-chunk tiles so compute can start
            # as soon as its own chunk lands).  Load order matters for the
            # startup bubble: zt(0) and memT (first GEMM's operands) go
            # first, split in halves to parallelize across DMA queues;
            # memB/colsum are only needed ~25us in.  zt tile 0 is
            # pre-issued here, before the memT flood.
            zt0 = zt_pool.tile([128, KC, 128], dt.bfloat16)
            nc.sync.dma_start(out=zt0, in_=zt_d[0])
            memt_sb = []
            for c in range(KC):
                tt = const_pool.tile([128, M], dt.bfloat16, tag=f"memt{c}")
                nc.gpsimd.dma_start(out=tt[:, 0:M // 2], in_=memt_d[:, c, 0:M // 2])
                nc.gpsimd.dma_start(out=tt[:, M // 2:], in_=memt_d[:, c, M // 2:])
                memt_sb.append(tt)
            memb_sb = []
            for c in range(JC):
                tb = const_pool.tile([128, F], dt.bfloat16, tag=f"memb{c}", name=f"memb{c}")
                nc.gpsimd.dma_start(out=tb, in_=memb_d[:, c, :])
                memb_sb.append(tb)
            colsum_sb = const_pool.tile([128, F], dt.float32, tag="colsum")
            nc.gpsimd.dma_start(out=colsum_sb, in_=colsum_d)

            # ---- software-pipelined main loop (GEMM2 skewed one tile) ----
            carry = None  # (vt_tile, r2_tile, t_index) awaiting GEMM2

            def gemm2_flush(carry):
                vt_t, r2, t = carry
                pz = psz_pool.tile([128, F], dt.float32, tag="pz")
                for c in range(JC):
                    for h in range(2):
                        nc.tensor.matmul(
                            pz[:, h * 512:(h + 1) * 512],
                            lhsT=vt_t[:, c, :],
                            rhs=memb_sb[c][:, h * 512:(h + 1) * 512],
                            start=(c == 0),
                            stop=(c == JC - 1),
                        )
                zh = zh_pool.tile([128, F], dt.float32)
                # zh = (pz + colsum) * r2   (add on DVE, scale on ACT)
                nc.vector.tensor_tensor(zh, pz, colsum_sb, op.add)
                nc.scalar.mul(zh, zh, r2)
                nc.sync.dma_start(out=zhat_d[t * 128:(t + 1) * 128, :], in_=zh)

            for t in range(T):
                if t == 0:
                    zt_t = zt0
                else:
                    zt_t = zt_pool.tile([128, KC, 128], dt.bfloat16)
                    nc.sync.dma_start(out=zt_t, in_=zt_d[t])

                # GEMM1 + exp, in quarters of 1024 logits.
                # Tile 0 runs chunk-major across all 4 quarters (8 PSUM
                # banks are free: no GEMM2 in flight yet), so PE consumes
                # memT chunk c as soon as it lands instead of needing the
                # whole 8 MiB before finishing quarter 0.
                u_t = u_pool.tile([128, M], dt.bfloat16)
                s_parts = stat_pool.tile([128, 4], dt.float32, tag="sparts")
                if t == 0:
                    pws = [
                        psw_pool.tile([128, 1024], dt.float32, tag="pw", name="pw0"),
                        psw_pool.tile([128, 1024], dt.float32, tag="pw", name="pw1"),
                        psz_pool.tile([128, 1024], dt.float32, tag="pz", name="pw2"),
                        psz_pool.tile([128, 1024], dt.float32, tag="pz", name="pw3"),
                    ]
                    for c in range(KC):
                        for q in range(4):
                            for h in range(2):
                                j0 = q * 1024 + h * 512
                                nc.tensor.matmul(
                                    pws[q][:, h * 512:(h + 1) * 512],
                                    lhsT=zt_t[:, c, :],
                                    rhs=memt_sb[c][:, j0:j0 + 512],
                                    start=(c == 0),
                                    stop=(c == KC - 1),
                                )
                    for q in range(4):
                        nc.scalar.activation(
                            u_t[:, q * 1024:(q + 1) * 1024],
                            pws[q],
                            afunc.Exp,
                            accum_out=s_parts[:, q:q + 1],
                        )
                else:
                    for q in range(4):
                        pw = psw_pool.tile([128, 1024], dt.float32, tag="pw")
                        for c in range(KC):
                            for h in range(2):
                                j0 = q * 1024 + h * 512
                                nc.tensor.matmul(
                                    pw[:, h * 512:(h + 1) * 512],
                                    lhsT=zt_t[:, c, :],
                                    rhs=memt_sb[c][:, j0:j0 + 512],
                                    start=(c == 0),
                                    stop=(c == KC - 1),
                                )
                        nc.scalar.activation(
                            u_t[:, q * 1024:(q + 1) * 1024],
                            pw,
                            afunc.Exp,
                            accum_out=s_parts[:, q:q + 1],
                        )

                # row stats: s, theta = lam*s, r = 1/s
                s = stat_pool.tile([128, 1], dt.float32, tag="s")
                nc.vector.reduce_sum(s, s_parts, axis=mybir.AxisListType.X)
                theta = stat_pool.tile([128, 1], dt.float32, tag="theta")
                nc.vector.tensor_scalar_mul(theta, s, LAM)
                r = stat_pool.tile([128, 1], dt.float32, tag="r")
                nc.vector.reciprocal(r, s)

                # um = (u > theta) * u   (in place on u)
                nc.vector.scalar_tensor_tensor(
                    u_t, u_t, theta, u_t, op.is_gt, op.mult
                )
                # v = um * r  (bf16), accumulate sv = rowsum(v)
                v_t = v_pool.tile([128, M], dt.bfloat16)
                sv = stat_pool.tile([128, 1], dt.float32, tag="sv")
                nc.vector.tensor_scalar(
                    v_t, u_t, r, 0.0, op.mult, op.add, accum_out=sv
                )
                # s2 = M + sv ; r2 = 1/s2
                s2 = stat_pool.tile([128, 1], dt.float32, tag="s2")
                nc.vector.tensor_scalar_add(s2, sv, float(M))
                r2 = stat_pool.tile([128, 1], dt.float32, tag="r2")
                nc.vector.reciprocal(r2, s2)

                # att = (v + 1) * r2, f32 out, DMA'd in quarters
                for h in range(4):
                    att_t = att_pool.tile([128, 1024], dt.float32)
                    nc.vector.tensor_scalar(
                        att_t, v_t[:, h * 1024:(h + 1) * 1024],
                        1.0, r2, op.add, op.mult,
                    )
                    nc.sync.dma_start(
                        out=att_d[t * 128:(t + 1) * 128, h * 1024:(h + 1) * 1024],
                        in_=att_t,
                    )

                # transpose v -> vt chunks in ONE xbar instruction:
                # vt[j, c, i] = v[i, c*128+j]
                vt_t = vt_pool.tile([128, JC, 128], dt.bfloat16)
                nc.sync.dma_start_transpose(vt_t, v_t)

                if carry is not None:
                    gemm2_flush(carry)
                carry = (vt_t, r2, t)
            gemm2_flush(carry)

    nc.compile()
    return nc


def _get_nc():
    if "nc" not in _STATE:
        _STATE["nc"] = _build_module()
    return _STATE["nc"]


def _prep_inputs(z: np.ndarray, memory: np.ndarray):
    mem_bf = memory.astype(BF16)
    memt_h = np.ascontiguousarray(
        mem_bf.T.reshape(KC, 128, M).transpose(1, 0, 2)
    )
    memb_h = np.ascontiguousarray(
        mem_bf.reshape(JC, 128, F).transpose(1, 0, 2)
    )
    colsum = memory.sum(axis=0, dtype=np.float64).astype(F32)
    colsum_h = np.ascontiguousarray(np.broadcast_to(colsum[None, :], (128, F)))

    z_bf = z.astype(BF16)
    in_maps = []
    for core in range(NCORES):
        zc = z_bf[core * BL:(core + 1) * BL]  # [2048, 1024]
        zt_h = np.ascontiguousarray(
            zc.reshape(T, 128, KC, 128).transpose(0, 3, 2, 1)
        )
        in_maps.append({
            "zt": zt_h,
            "memt": memt_h,
            "memb": memb_h,
            "colsum": colsum_h,
        })
    return in_maps


def kernel(z: np.ndarray, memory: np.ndarray, _trace: bool = False):
    nc = _get_nc()
    in_maps = _prep_inputs(np.asarray(z, dtype=F32), np.asarray(memory, dtype=F32))
    res = run_bass_kernel_spmd(
        nc, in_maps, core_ids=list(range(NCORES)), trace=_trace
    )
    _STATE["last_result"] = res
    att = np.concatenate([res.results[i]["att"] for i in range(NCORES)], axis=0)
    zhat = np.concatenate([res.results[i]["zhat"] for i in range(NCORES)], axis=0)
    return att, zhat
